# revision 17
# baseline (speedup 1.0000x reference)
"""Biaffine kernel for Trainium2 (8 NeuronCores, SPMD batch-parallel).

Computes, for inputs input1/input2 (B=32, S=1024, D=256), w1 (D, O=2, D),
w2 (2D+1, O):

    out[b,x,y,o] = sum_ij input1[b,x,i] * w1[i,o,j] * input2[b,y,j]
                 + input1[b,x,:] @ w2[:D, o]   (lin1, folded into evac bias)
                 + input2[b,y,:] @ w2[D:2D, o] (lin2, folded into UT on host)
                 + w2[2D, o]                   (bias, folded with lin1)

Split of work:
  host:   UT[b][j, o, jt, x] = sum_i w1[i,o,j]*input1[b,x,i] + w2[D+j,o]
          (8.6 GFLOP fp32 BLAS, then rounded to fp16)
  device: out[x, y] = sum_j UT[o][j, x] * in2T[j, y]   (PE, fp16 operands,
          fp32 PSUM accumulation), + per-partition bias (lin1[x,o]+w2[2D,o])
          applied during the PSUM->SBUF evacuation, output staged as fp16.

Sharding: batch (32) split 4-per-core across 8 cores, no collectives.

v7 structure (evolved from the v1 baseline via per-instruction traces):
  - 8 warm-up matmuls on a zeroed tile right after the framework preamble
    flip the PE HAM clock gate (1.2 -> 2.4 GHz) before real matmuls start
  - loads use whole-tile DMAs only (>=4 KiB contiguous runs per partition;
    sub-1KiB runs transfer 2x slower), split across the two HWDGE rings:
    scalar ring takes ut (x-half-major so each half is one 4KiB-run DMA),
    sync ring takes in2t + lina; stores own the sync ring afterwards
  - b2/b3 bulk prefetch on the gpsimd SWDGE ring mid-batch-0, so it never
    competes with the critical head loads
  - evacuation: whole [128, S] PSUM tile per (xt, o), alternating engines
    (ACT even groups / DVE odd) -- two engines writing one out tile are
    serialized by Tile, so halves must NOT be split across engines
  - jt-outer matmul order (consecutive matmuls share lhsT weights)
"""

import os
import sys

for _p in ("/opt/trn_rl_repo",):
    if _p not in sys.path and os.path.isdir(_p):
        sys.path.insert(0, _p)

import numpy as np

B, S, D, O = 32, 1024, 256, 2
NCORES = 8
BP = B // NCORES          # batches per core
XT = S // 128             # x tiles per batch
XH = S // 2               # x-half size (512)
NSL = 512                 # matmul moving free dim (one PSUM bank of fp32)

_nc_cache = {}
last_results = None       # BassKernelResults of the most recent run (for test.py)


def _build_nc():
    import concourse.bass as bass
    import concourse.mybir as mybir
    import concourse.tile as tile
    from concourse import bacc

    f32 = mybir.dt.float32
    f16 = mybir.dt.float16
    AF = mybir.ActivationFunctionType

    nc = bacc.Bacc(None, target_bir_lowering=False, debug=False)

    # partition-major DRAM layouts; ut additionally x-half-major so each
    # half-load is 128 contiguous 4 KiB runs
    ut_d = nc.dram_tensor("ut", [BP, 2, 128, O, 2, XH], f16, kind="ExternalInput")
    in2t_d = nc.dram_tensor("in2t", [BP, 128, 2, S], f16, kind="ExternalInput")
    lina_d = nc.dram_tensor("lina", [128, BP, O, XT], f32, kind="ExternalInput")
    out_d = nc.dram_tensor("out", [BP, XT, O, 128, S], f16, kind="ExternalOutput")

    with tile.TileContext(nc) as tc:
        with (
            tc.tile_pool(name="const", bufs=1) as cpool,
            tc.tile_pool(name="inp", bufs=4) as ipool,
            tc.tile_pool(name="outp", bufs=6) as opool,
            # PSUM split by evacuating engine: pool A tiles are always
            # evacuated by ScalarE, pool B tiles by VectorE
            tc.tile_pool(name="psumA", bufs=2, space=bass.MemorySpace.PSUM) as ppoolA,
            tc.tile_pool(name="psumB", bufs=2, space=bass.MemorySpace.PSUM) as ppoolB,
        ):
            lina_sb = cpool.tile([128, BP, O, XT], f32, tag="lina_sb")
            warm_sb = cpool.tile([128, NSL], f16, tag="warm_sb")

            # ---- PE warm-up ---------------------------------------------
            nc.vector.memset(warm_sb[:], 0.0)
            warm_ps = ppoolA.tile([128, S], f32, tag="psum_a")
            for _ in range(6):
                nc.tensor.matmul(
                    warm_ps[:, 0:NSL], lhsT=warm_sb[:, 0:128], rhs=warm_sb[:],
                    start=True, stop=True,
                )

            # ---- input loads --------------------------------------------
            ut_tiles, in2_tiles = [], []

            def alloc_b():
                # x-halves as SEPARATE tiles: Tile dependency tracking is
                # tile-granular, so one tile fed by two DMAs would make
                # every reader wait for the LAST write
                ut_a = ipool.tile([128, O, 2, XH], f16, tag="ut_a")
                ut_b = ipool.tile([128, O, 2, XH], f16, tag="ut_b")
                in2_j0 = ipool.tile([128, S], f16, tag="in2_j0")
                in2_j1 = ipool.tile([128, S], f16, tag="in2_j1")
                ut_tiles.append((ut_a, ut_b))
                in2_tiles.append((in2_j0, in2_j1))
                return ut_a, ut_b, in2_j0, in2_j1

            ut0a, ut0b, in20j0, in20j1 = alloc_b()
            nc.scalar.dma_start(out=ut0a[:], in_=ut_d[0, 0])
            nc.sync.dma_start(out=in20j0[:], in_=in2t_d[0, :, 0])
            nc.sync.dma_start(out=in20j1[:], in_=in2t_d[0, :, 1])
            nc.scalar.dma_start(out=ut0b[:], in_=ut_d[0, 1])
            nc.sync.dma_start(out=lina_sb[:], in_=lina_d[:])
            ut1a, ut1b, in21j0, in21j1 = alloc_b()
            nc.scalar.dma_start(out=ut1a[:], in_=ut_d[1, 0])
            nc.sync.dma_start(out=in21j0[:], in_=in2t_d[1, :, 0])
            nc.scalar.dma_start(out=ut1b[:], in_=ut_d[1, 1])
            nc.sync.dma_start(out=in21j1[:], in_=in2t_d[1, :, 1])
            for b in range(2, BP):
                alloc_b()

            def evac_act(dst, src, bias):
                nc.scalar.activation(dst, src, AF.Identity, bias=bias, scale=1.0)

            def evac_dve(dst, src, bias):
                nc.vector.tensor_scalar(
                    out=dst, in0=src, scalar1=bias, scalar2=None,
                    op0=mybir.AluOpType.add,
                )

            # ---- compute + evac + store ---------------------------------
            for b in range(BP):
                uts, in2s = ut_tiles[b], in2_tiles[b]
                for xt in range(XT):
                    # bulk prefetch for b2/b3 on the SWDGE ring, slotted
                    # mid-batch-0 (after the critical head loads are done)
                    if b == 0 and xt == 2:
                        nc.gpsimd.dma_start(out=ut_tiles[2][0][:], in_=ut_d[2, 0])
                        nc.gpsimd.dma_start(out=ut_tiles[2][1][:], in_=ut_d[2, 1])
                        nc.gpsimd.dma_start(out=in2_tiles[2][0][:], in_=in2t_d[2, :, 0])
                        nc.gpsimd.dma_start(out=in2_tiles[2][1][:], in_=in2t_d[2, :, 1])
                    if b == 0 and xt == 5:
                        nc.gpsimd.dma_start(out=ut_tiles[3][0][:], in_=ut_d[3, 0])
                        nc.gpsimd.dma_start(out=ut_tiles[3][1][:], in_=ut_d[3, 1])
                        nc.gpsimd.dma_start(out=in2_tiles[3][0][:], in_=in2t_d[3, :, 0])
                        nc.gpsimd.dma_start(out=in2_tiles[3][1][:], in_=in2t_d[3, :, 1])
                    ut_h, xi = uts[xt // 4], xt % 4
                    for o in range(O):
                        use_a = (xt * 2 + o) % 2 == 0
                        ps = (ppoolA if use_a else ppoolB).tile(
                            [128, S], f32, tag="psum_a" if use_a else "psum_b")
                        # jt-outer: consecutive matmuls share lhsT
                        for jt in range(2):
                            for yn in range(2):
                                nc.tensor.matmul(
                                    ps[:, yn * NSL:(yn + 1) * NSL],
                                    lhsT=ut_h[:, o, jt, xi * 128:(xi + 1) * 128],
                                    rhs=in2s[jt][:, yn * NSL:(yn + 1) * NSL],
                                    start=(jt == 0), stop=(jt == 1),
                                )
                        # per-(xt,o) out tile: ACT and DVE never write the
                        # same tile, so their evacs run truly concurrent
                        out_sb = opool.tile(
                            [128, S], f16, tag="out_a" if use_a else "out_b")
                        ev = evac_act if use_a else evac_dve
                        ev(out_sb[:], ps[:], lina_sb[:, b, o, xt:xt + 1])
                        nc.sync.dma_start(out=out_d[b, xt, o], in_=out_sb[:])

    nc.compile()
    return nc


def kernel(input1, input2, w1, w2):
    global last_results
    from concourse.bass_utils import run_bass_kernel_spmd

    input1 = np.ascontiguousarray(input1, dtype=np.float32)
    input2 = np.ascontiguousarray(input2, dtype=np.float32)
    w1 = np.ascontiguousarray(w1, dtype=np.float32)
    w2 = np.ascontiguousarray(w2, dtype=np.float32)

    # host stage 1: u[b,x,o,j] = sum_i input1[b,x,i] w1[i,o,j] + w2[D+j,o]
    u = (input1.reshape(B * S, D) @ w1.reshape(D, O * D)).reshape(B, S, O, D)
    u += w2[D:2 * D].T[None, None, :, :]          # fold lin2 weights
    # device layout [b, xh, j128, o, jt, x-in-half]
    ut = np.ascontiguousarray(
        u.transpose(0, 3, 2, 1)                    # (B, D, O, S)
        .reshape(B, 2, 128, O, 2, XH)              # D->(jt,j128), S->(xh,xx)
        .transpose(0, 4, 2, 3, 1, 5),              # (B, 2, 128, O, 2, XH)
        dtype=np.float16)

    # transposed input2 -> [B, j128, jt, S] fp16
    in2t = np.ascontiguousarray(
        input2.transpose(0, 2, 1)                  # (B, D, S)
        .reshape(B, 2, 128, S)                     # D -> (jt, j128)
        .transpose(0, 2, 1, 3),                    # (B, 128, 2, S)
        dtype=np.float16)

    # lin1 + bias: (B, S, O) -> per-core [x128, b, o, xt], fp32
    lina = input1 @ w2[:D] + w2[2 * D]
    lina_dev = np.ascontiguousarray(
        lina.reshape(B, XT, 128, O).transpose(2, 0, 3, 1)
    )  # (128, B, O, XT)

    in_maps = []
    for c in range(NCORES):
        bs = slice(c * BP, (c + 1) * BP)
        in_maps.append({
            "ut": np.ascontiguousarray(ut[bs]),
            "in2t": np.ascontiguousarray(in2t[bs]),
            "lina": np.ascontiguousarray(lina_dev[:, bs]),
        })

    if "nc" not in _nc_cache:
        _nc_cache["nc"] = _build_nc()
    nc = _nc_cache["nc"]

    trace = bool(int(os.environ.get("BIAFFINE_TRACE", "0")))
    if trace:
        _install_ntff_hook_shim()

    res = run_bass_kernel_spmd(
        nc, in_maps, core_ids=list(range(NCORES)), trace=trace,
        trace_cores=list(range(NCORES)) if trace else None,
        stitch_traces=False,
    )
    last_results = res

    out = np.empty((B, S, S, O), dtype=np.float32)
    for c in range(NCORES):
        dev = res.results[c]["out"]  # (BP, XT, O, 128, S) fp16
        # -> (BP, XT, 128, S, O) -> (BP, S, S, O), upcast to fp32
        out[c * BP:(c + 1) * BP] = (
            dev.transpose(0, 1, 3, 4, 2).reshape(BP, S, S, O).astype(np.float32)
        )
    return out


def _install_ntff_hook_shim():
    """Register the axon NTFF profiling hook (the container's antenv stub
    lacks axon_hooks, so trn_boot's registration degraded silently)."""
    import types
    try:
        from antenv.axon_hooks import get_axon_ntff_profile_hook  # noqa: F401
        return  # already present
    except ImportError:
        pass
    import antenv
    mod = types.ModuleType("antenv.axon_hooks")
    _hook = [None]
    mod.set_axon_ntff_profile_hook = lambda h: _hook.__setitem__(0, h)
    mod.get_axon_ntff_profile_hook = lambda: _hook[0]
    sys.modules["antenv.axon_hooks"] = mod
    antenv.axon_hooks = mod
    try:
        from trn_agent_boot.trn_boot import _ntff_profile_via_ctypes
        so_path = "/opt/axon/libaxon_pjrt.so"
        if os.path.exists(so_path):
            mod.set_axon_ntff_profile_hook(_ntff_profile_via_ctypes(so_path))
    except Exception:
        pass


# revision 18
# speedup vs baseline: 1.0361x; 1.0361x over previous
"""Biaffine kernel for Trainium2 (8 NeuronCores, SPMD batch-parallel).

Computes, for inputs input1/input2 (B=32, S=1024, D=256), w1 (D, O=2, D),
w2 (2D+1, O):

    out[b,x,y,o] = sum_ij input1[b,x,i] * w1[i,o,j] * input2[b,y,j]
                 + input1[b,x,:] @ w2[:D, o]   (lin1, folded into evac bias)
                 + input2[b,y,:] @ w2[D:2D, o] (lin2, folded into UT on host)
                 + w2[2D, o]                   (bias, folded with lin1)

Split of work:
  host:   UT[b][j, o, jt, x] = sum_i w1[i,o,j]*input1[b,x,i] + w2[D+j,o]
          (8.6 GFLOP fp32 BLAS, then rounded to fp16)
  device: out[x, y] = sum_j UT[o][j, x] * in2T[j, y]   (PE, fp16 operands,
          fp32 PSUM accumulation), + per-partition bias (lin1[x,o]+w2[2D,o])
          applied during the PSUM->SBUF evacuation, output staged as fp16.

Sharding: batch (32) split 4-per-core across 8 cores, no collectives.

v7 structure (evolved from the v1 baseline via per-instruction traces):
  - 8 warm-up matmuls on a zeroed tile right after the framework preamble
    flip the PE HAM clock gate (1.2 -> 2.4 GHz) before real matmuls start
  - loads use whole-tile DMAs only (>=4 KiB contiguous runs per partition;
    sub-1KiB runs transfer 2x slower), split across the two HWDGE rings:
    scalar ring takes ut (x-half-major so each half is one 4KiB-run DMA),
    sync ring takes in2t + lina; stores own the sync ring afterwards
  - b2/b3 bulk prefetch on the gpsimd SWDGE ring mid-batch-0, so it never
    competes with the critical head loads
  - evacuation: whole [128, S] PSUM tile per (xt, o), alternating engines
    (ACT even groups / DVE odd) -- two engines writing one out tile are
    serialized by Tile, so halves must NOT be split across engines
  - jt-outer matmul order (consecutive matmuls share lhsT weights)
"""

import os
import sys

for _p in ("/opt/trn_rl_repo",):
    if _p not in sys.path and os.path.isdir(_p):
        sys.path.insert(0, _p)

import numpy as np

B, S, D, O = 32, 1024, 256, 2
NCORES = 8
BP = B // NCORES          # batches per core
XT = S // 128             # x tiles per batch
XH = S // 2               # x-half size (512)
NSL = 512                 # matmul moving free dim (one PSUM bank of fp32)

_nc_cache = {}
last_results = None       # BassKernelResults of the most recent run (for test.py)


def _build_nc():
    import concourse.bass as bass
    import concourse.mybir as mybir
    import concourse.tile as tile
    from concourse import bacc

    f32 = mybir.dt.float32
    f16 = mybir.dt.float16
    AF = mybir.ActivationFunctionType

    nc = bacc.Bacc(None, target_bir_lowering=False, debug=False)

    # partition-major DRAM layouts; ut additionally x-half-major so each
    # half-load is 128 contiguous 4 KiB runs
    ut_d = nc.dram_tensor("ut", [BP, 2, 128, O, 2, XH], f16, kind="ExternalInput")
    in2t_d = nc.dram_tensor("in2t", [BP, 128, 2, S], f16, kind="ExternalInput")
    lina_d = nc.dram_tensor("lina", [128, BP, O, XT], f32, kind="ExternalInput")
    out_d = nc.dram_tensor("out", [BP, XT, 128, O, S], f16, kind="ExternalOutput")

    with tile.TileContext(nc) as tc:
        with (
            tc.tile_pool(name="const", bufs=1) as cpool,
            tc.tile_pool(name="inp", bufs=4) as ipool,
            tc.tile_pool(name="outp", bufs=6) as opool,
            # PSUM split by evacuating engine: pool A tiles are always
            # evacuated by ScalarE, pool B tiles by VectorE
            tc.tile_pool(name="psumA", bufs=2, space=bass.MemorySpace.PSUM) as ppoolA,
            tc.tile_pool(name="psumB", bufs=2, space=bass.MemorySpace.PSUM) as ppoolB,
        ):
            lina_sb = cpool.tile([128, BP, O, XT], f32, tag="lina_sb")
            warm_sb = cpool.tile([128, NSL], f16, tag="warm_sb")

            # ---- PE warm-up ---------------------------------------------
            nc.vector.memset(warm_sb[:], 0.0)
            warm_ps = ppoolA.tile([128, S], f32, tag="psum_a")
            for _ in range(8):
                nc.tensor.matmul(
                    warm_ps[:, 0:NSL], lhsT=warm_sb[:, 0:128], rhs=warm_sb[:],
                    start=True, stop=True,
                )

            # ---- input loads --------------------------------------------
            ut_tiles, in2_tiles = [], []

            def alloc_b():
                ut_sb = ipool.tile([128, 2, O, 2, XH], f16, tag="ut_sb")
                in2_sb = ipool.tile([128, 2, S], f16, tag="in2_sb")
                ut_tiles.append(ut_sb)
                in2_tiles.append(in2_sb)
                return ut_sb, in2_sb

            ut0, in20 = alloc_b()
            nc.scalar.dma_start(out=ut0[:, 0], in_=ut_d[0, 0])
            nc.sync.dma_start(out=in20[:], in_=in2t_d[0])
            nc.scalar.dma_start(out=ut0[:, 1], in_=ut_d[0, 1])
            nc.sync.dma_start(out=lina_sb[:], in_=lina_d[:])
            ut1, in21 = alloc_b()
            nc.scalar.dma_start(out=ut1[:, 0], in_=ut_d[1, 0])
            nc.sync.dma_start(out=in21[:], in_=in2t_d[1])
            nc.scalar.dma_start(out=ut1[:, 1], in_=ut_d[1, 1])
            for b in range(2, BP):
                alloc_b()

            def evac_act(dst, src, bias):
                nc.scalar.activation(dst, src, AF.Identity, bias=bias, scale=1.0)

            def evac_dve(dst, src, bias):
                nc.vector.tensor_scalar(
                    out=dst, in0=src, scalar1=bias, scalar2=None,
                    op0=mybir.AluOpType.add,
                )

            # ---- compute + evac + store ---------------------------------
            for b in range(BP):
                ut_sb, in2_sb = ut_tiles[b], in2_tiles[b]
                for xt in range(XT):
                    # bulk prefetch for b2/b3 on the SWDGE ring, slotted
                    # mid-batch-0 (after the critical head loads are done)
                    if b == 0 and xt == 2:
                        nc.gpsimd.dma_start(out=ut_tiles[2][:, 0], in_=ut_d[2, 0])
                        nc.gpsimd.dma_start(out=ut_tiles[2][:, 1], in_=ut_d[2, 1])
                        nc.gpsimd.dma_start(out=in2_tiles[2][:], in_=in2t_d[2])
                    if b == 0 and xt == 5:
                        nc.gpsimd.dma_start(out=ut_tiles[3][:, 0], in_=ut_d[3, 0])
                        nc.gpsimd.dma_start(out=ut_tiles[3][:, 1], in_=ut_d[3, 1])
                        nc.gpsimd.dma_start(out=in2_tiles[3][:], in_=in2t_d[3])
                    xh, xi = xt // 4, xt % 4
                    out_sb = opool.tile([128, O, S], f16, tag="out_sb")
                    for o in range(O):
                        use_a = (xt * 2 + o) % 2 == 0
                        ps = (ppoolA if use_a else ppoolB).tile(
                            [128, S], f32, tag="psum_a" if use_a else "psum_b")
                        # jt-outer: consecutive matmuls share lhsT
                        for jt in range(2):
                            for yn in range(2):
                                nc.tensor.matmul(
                                    ps[:, yn * NSL:(yn + 1) * NSL],
                                    lhsT=ut_sb[:, xh, o, jt, xi * 128:(xi + 1) * 128],
                                    rhs=in2_sb[:, jt, yn * NSL:(yn + 1) * NSL],
                                    start=(jt == 0), stop=(jt == 1),
                                )
                        ev = evac_act if use_a else evac_dve
                        ev(out_sb[:, o], ps[:], lina_sb[:, b, o, xt:xt + 1])
                    nc.sync.dma_start(out=out_d[b, xt], in_=out_sb[:])

    nc.compile()
    return nc


def kernel(input1, input2, w1, w2):
    global last_results
    from concourse.bass_utils import run_bass_kernel_spmd

    input1 = np.ascontiguousarray(input1, dtype=np.float32)
    input2 = np.ascontiguousarray(input2, dtype=np.float32)
    w1 = np.ascontiguousarray(w1, dtype=np.float32)
    w2 = np.ascontiguousarray(w2, dtype=np.float32)

    # host stage 1: u[b,x,o,j] = sum_i input1[b,x,i] w1[i,o,j] + w2[D+j,o]
    u = (input1.reshape(B * S, D) @ w1.reshape(D, O * D)).reshape(B, S, O, D)
    u += w2[D:2 * D].T[None, None, :, :]          # fold lin2 weights
    # device layout [b, xh, j128, o, jt, x-in-half]
    ut = np.ascontiguousarray(
        u.transpose(0, 3, 2, 1)                    # (B, D, O, S)
        .reshape(B, 2, 128, O, 2, XH)              # D->(jt,j128), S->(xh,xx)
        .transpose(0, 4, 2, 3, 1, 5),              # (B, 2, 128, O, 2, XH)
        dtype=np.float16)

    # transposed input2 -> [B, j128, jt, S] fp16
    in2t = np.ascontiguousarray(
        input2.transpose(0, 2, 1)                  # (B, D, S)
        .reshape(B, 2, 128, S)                     # D -> (jt, j128)
        .transpose(0, 2, 1, 3),                    # (B, 128, 2, S)
        dtype=np.float16)

    # lin1 + bias: (B, S, O) -> per-core [x128, b, o, xt], fp32
    lina = input1 @ w2[:D] + w2[2 * D]
    lina_dev = np.ascontiguousarray(
        lina.reshape(B, XT, 128, O).transpose(2, 0, 3, 1)
    )  # (128, B, O, XT)

    in_maps = []
    for c in range(NCORES):
        bs = slice(c * BP, (c + 1) * BP)
        in_maps.append({
            "ut": np.ascontiguousarray(ut[bs]),
            "in2t": np.ascontiguousarray(in2t[bs]),
            "lina": np.ascontiguousarray(lina_dev[:, bs]),
        })

    if "nc" not in _nc_cache:
        _nc_cache["nc"] = _build_nc()
    nc = _nc_cache["nc"]

    trace = bool(int(os.environ.get("BIAFFINE_TRACE", "0")))
    if trace:
        _install_ntff_hook_shim()

    res = run_bass_kernel_spmd(
        nc, in_maps, core_ids=list(range(NCORES)), trace=trace,
        trace_cores=list(range(NCORES)) if trace else None,
        stitch_traces=False,
    )
    last_results = res

    out = np.empty((B, S, S, O), dtype=np.float32)
    for c in range(NCORES):
        dev = res.results[c]["out"]  # (BP, XT, 128, O, S) fp16
        # -> (BP, XT, 128, S, O) -> (BP, S, S, O), upcast to fp32
        out[c * BP:(c + 1) * BP] = (
            dev.transpose(0, 1, 2, 4, 3).reshape(BP, S, S, O).astype(np.float32)
        )
    return out


def _install_ntff_hook_shim():
    """Register the axon NTFF profiling hook (the container's antenv stub
    lacks axon_hooks, so trn_boot's registration degraded silently)."""
    import types
    try:
        from antenv.axon_hooks import get_axon_ntff_profile_hook  # noqa: F401
        return  # already present
    except ImportError:
        pass
    import antenv
    mod = types.ModuleType("antenv.axon_hooks")
    _hook = [None]
    mod.set_axon_ntff_profile_hook = lambda h: _hook.__setitem__(0, h)
    mod.get_axon_ntff_profile_hook = lambda: _hook[0]
    sys.modules["antenv.axon_hooks"] = mod
    antenv.axon_hooks = mod
    try:
        from trn_agent_boot.trn_boot import _ntff_profile_via_ctypes
        so_path = "/opt/axon/libaxon_pjrt.so"
        if os.path.exists(so_path):
            mod.set_axon_ntff_profile_hook(_ntff_profile_via_ctypes(so_path))
    except Exception:
        pass
